# revision 12
# baseline (speedup 1.0000x reference)
"""Trainium2 Bass kernel for CoExDispProcessor (topk_masking).

Per-sample computation (data-parallel over batch across 8 cores):
  1. top-2 over the D=48 disparity axis of cost [1,48,128,240] -> softmax
     blend of the two indices -> disp4 [128,240]
  2. 3x3 unfold of disp4 (zero pad) -> nearest 4x upsample -> weighted sum
     with softmax over the 9 channels of spg [9,512,960] -> disp1 [512,960]

Kernel layout choices:
  - top-2: cost DMA'd into [128(h), 48(d), wchunk] tiles; vector.max /
    max_index (top-8 values+indices) per w column on strided [128, 48] APs.
  - fine stage: partition = coarse row R (128 of them), free = (dr, w).
    exp on ACT (bf16 out), 9-channel weighted accumulation on DVE with the
    coarse patch values broadcast via stride-0 APs.
"""

import os
import sys
from contextlib import ExitStack

import numpy as np

if "/opt/trn_rl_repo" not in sys.path:
    sys.path.insert(0, "/opt/trn_rl_repo")

import concourse.bass as bass
import concourse.bacc as bacc
import concourse.tile as tile
from concourse import mybir
from concourse.bass_utils import run_bass_kernel_spmd

F32 = mybir.dt.float32
BF16 = mybir.dt.bfloat16
FP16 = mybir.dt.float16
U16 = mybir.dt.uint16
OP = mybir.AluOpType
ACT = mybir.ActivationFunctionType

B, D, H, W = 8, 48, 128, 240
NPIX = H * W  # 30720
HF, WF = 4 * H, 4 * W  # 512, 960
N_CORES = 8

# top-2 phase tiling: ragged chunks of w columns (small first chunk so the
# max/max_index stream starts as early as possible)
COST_CHUNKS = [12, 48, 60, 60, 60]

# fine phase tiling: split the 960 fine columns into chunks
FINE_CHUNK = 480  # fine columns per chunk
N_FINE_CHUNKS = WF // FINE_CHUNK  # 4
WC = FINE_CHUNK // 4  # coarse columns per chunk (60)


def _act_reciprocal(nc, out_ap, in_ap):
    eng = nc.scalar
    return eng.add_instruction(
        mybir.InstActivation(
            name=nc.get_next_instruction_name(),
            func=ACT.Reciprocal,
            ins=[
                eng.lower_ap(in_ap),
                mybir.ImmediateValue(dtype=F32, value=0.0),
                mybir.ImmediateValue(dtype=F32, value=1.0),
                mybir.ImmediateValue(dtype=F32, value=0.0),
            ],
            outs=[eng.lower_ap(out_ap)],
        )
    )


def build_kernel(ctx: ExitStack, tc: tile.TileContext, out_d, cost_d, spg_d):
    nc = tc.nc

    cost_hdw = cost_d.transpose([1, 0, 2])  # [128(h), 48(d), 240(w)] view
    # [c, R, dr, k, w] view of spg / out for partition=R fine layout
    spg_v = spg_d.rearrange(
        "c (R dr) (k w) -> c R dr k w", dr=4, k=N_FINE_CHUNKS
    )
    out_v = out_d.rearrange("(R dr) (k w) -> R dr k w", dr=4, k=N_FINE_CHUNKS)

    persist = ctx.enter_context(tc.tile_pool(name="persist", bufs=1))
    cost_pool = ctx.enter_context(tc.tile_pool(name="costc", bufs=2))
    small = ctx.enter_context(tc.tile_pool(name="small", bufs=1))
    raw_pool = ctx.enter_context(tc.tile_pool(name="raw", bufs=3))
    e_pool = ctx.enter_context(tc.tile_pool(name="epool", bufs=12))
    den_pool = ctx.enter_context(tc.tile_pool(name="den", bufs=1))
    prod_pool = ctx.enter_context(tc.tile_pool(name="prod", bufs=9))
    fin_pool = ctx.enter_context(tc.tile_pool(name="fin", bufs=1))
    out_pool = ctx.enter_context(tc.tile_pool(name="outp", bufs=2))

    # ---------------- top-2 phase ----------------------------------------
    v8 = persist.tile([128, W, 8], F32)
    i8 = persist.tile([128, W, 8], U16)
    w0 = 0
    for nw in COST_CHUNKS:
        ctile = cost_pool.tile([128, D, 60], F32, tag="cost")
        nc.sync.dma_start(ctile[:, :, :nw], cost_hdw[:, :, w0:w0 + nw])
        for j in range(nw):
            nc.vector.max(out=v8[:, w0 + j], in_=ctile[:, :, j])
        for j in range(nw):
            w = w0 + j
            nc.vector.max_index(i8[:, w], v8[:, w], ctile[:, :, j])
        w0 += nw

    # ---------------- disp4 from top-2 ------------------------------------
    i1f = small.tile([128, W], F32)
    i2f = small.tile([128, W], F32)
    delta = small.tile([128, W], F32)
    texp = small.tile([128, W], F32)
    numc = small.tile([128, W], F32)
    denc = small.tile([128, W], F32)
    rden = small.tile([128, W], F32)
    disp4 = small.tile([128, W], F32)

    nc.vector.tensor_copy(i1f[:], i8[:, :, 0])
    nc.vector.tensor_copy(i2f[:], i8[:, :, 1])
    nc.vector.tensor_sub(delta[:], v8[:, :, 1], v8[:, :, 0])
    nc.scalar.activation(texp[:], delta[:], ACT.Exp)
    nc.vector.tensor_mul(numc[:], texp[:], i2f[:])
    nc.vector.tensor_add(numc[:], numc[:], i1f[:])
    nc.vector.tensor_scalar_add(denc[:], texp[:], 1.0)
    nc.vector.reciprocal(rden[:], denc[:])
    nc.vector.tensor_mul(disp4[:], numc[:], rden[:])

    # ---------------- patches: 3 row-shifted, col-padded variants ---------
    # rv[s][r, 1+w] = disp4[r + s - 1, w]  (zero outside), s in {0,1,2}
    rv = []
    for s in range(3):
        t = small.tile([128, W + 2], F32, tag=f"rv{s}")
        rv.append(t)
        nc.vector.memset(t[:], 0.0)
    nc.vector.tensor_copy(rv[1][:, 1:W + 1], disp4[:])
    nc.sync.dma_start(rv[0][1:128, 1:W + 1], disp4[0:127, :])
    nc.sync.dma_start(rv[2][0:127, 1:W + 1], disp4[1:128, :])
    # u_rep[s][r, 4*x + dw] = rv[s][r, x]  (fp16, fine-resolution replication)
    urep = []
    for s in range(3):
        t = small.tile([128, 4 * (W + 2)], FP16, tag=f"urep{s}")
        urep.append(t)
        nc.scalar.copy(
            t[:].rearrange("p (x dw) -> p x dw", dw=4),
            rv[s].unsqueeze(2).broadcast_to([128, W + 2, 4]),
        )

    # ---------------- fine stage part 1: load spg, exp, denominator ------
    # (independent of the top-2 phase -> overlaps it)
    e_tiles = {}
    den_tiles = []
    for k in range(N_FINE_CHUNKS):
        den = den_pool.tile([128, 4 * FINE_CHUNK], FP16, tag="den")
        den_tiles.append(den)
        for c in range(9):
            raw = raw_pool.tile([128, 4, FINE_CHUNK], F32, tag="raw")
            nc.sync.dma_start(raw[:], spg_v[c, :, :, k, :])
            e = e_pool.tile([128, 4, FINE_CHUNK], FP16, tag="e")
            e_tiles[(k, c)] = e
            nc.scalar.activation(e[:], raw[:], ACT.Exp)
            ef = e[:].rearrange("p a b -> p (a b)")
            if c == 0:
                nc.gpsimd.tensor_copy(den[:], ef)
            else:
                nc.gpsimd.tensor_add(den[:], den[:], ef)

    # ---------------- fine stage part 2: numerator, divide, store ---------
    for k in range(N_FINE_CHUNKS):
        prods = []
        for c in range(9):
            cirow, cj = c // 3, c % 3
            off = 4 * (cj + k * WC)
            u4 = (urep[cirow][:, off:off + FINE_CHUNK]
                  .unsqueeze(1).broadcast_to([128, 4, FINE_CHUNK]))
            ev = e_tiles[(k, c)][:]
            p = prod_pool.tile([128, 4 * FINE_CHUNK], FP16, tag="prod")
            nc.vector.tensor_mul(p[:].rearrange("p (a b) -> p a b", a=4), ev, u4)
            prods.append(p)
        # in-place pairwise tree: adjacent instructions stay independent
        for a, b in ((0, 1), (2, 3), (4, 5), (6, 7)):
            nc.vector.tensor_add(prods[a][:], prods[a][:], prods[b][:])
        for a, b in ((0, 2), (4, 6)):
            nc.vector.tensor_add(prods[a][:], prods[a][:], prods[b][:])
        nc.vector.tensor_add(prods[0][:], prods[0][:], prods[4][:])
        nc.vector.tensor_add(prods[0][:], prods[0][:], prods[8][:])
        num = prods[0]
        den = den_tiles[k]
        r0 = fin_pool.tile([128, 4 * FINE_CHUNK], FP16, tag="r0")
        _act_reciprocal(nc, r0[:], den[:])
        m = fin_pool.tile([128, 4 * FINE_CHUNK], FP16, tag="m")
        nc.vector.tensor_mul(m[:], den[:], r0[:])
        nc.vector.tensor_scalar(m[:], m[:], -1.0, 2.0, op0=OP.mult, op1=OP.add)
        rdf = fin_pool.tile([128, 4 * FINE_CHUNK], FP16, tag="rdf")
        nc.vector.tensor_mul(rdf[:], r0[:], m[:])
        outt = out_pool.tile([128, 4, FINE_CHUNK], F32, tag="outt")
        nc.vector.scalar_tensor_tensor(
            outt[:].rearrange("p a b -> p (a b)"), num[:], 4.0, rdf[:],
            op0=OP.mult, op1=OP.mult,
        )
        nc.sync.dma_start(out_v[:, :, k, :], outt[:])


def build_program():
    nc = bacc.Bacc(
        "TRN2",
        target_bir_lowering=False,
        debug=False,
        enable_asserts=False,
        num_devices=N_CORES,
    )
    cost_d = nc.dram_tensor("cost", [D, H, W], F32, kind="ExternalInput").ap()
    spg_d = nc.dram_tensor("spg", [9, HF, WF], F32, kind="ExternalInput").ap()
    out_d = nc.dram_tensor("out", [HF, WF], F32, kind="ExternalOutput").ap()
    with tile.TileContext(nc) as tc:
        with ExitStack() as ctx:
            build_kernel(ctx, tc, out_d, cost_d, spg_d)
    nc.compile()
    return nc


def _install_ntff_hook():
    """Provide antenv.axon_hooks + register the ctypes NTFF profiler.

    The agent image's antenv package lacks axon_hooks, so trn_boot's hook
    registration silently degraded.  Recreate both pieces here.
    """
    import types

    if "antenv.axon_hooks" in sys.modules:
        return True
    try:
        import antenv
        from trn_agent_boot.trn_boot import _ntff_profile_via_ctypes

        mod = types.ModuleType("antenv.axon_hooks")
        mod._hook = None

        def set_axon_ntff_profile_hook(hook):
            mod._hook = hook

        def get_axon_ntff_profile_hook():
            return mod._hook

        mod.set_axon_ntff_profile_hook = set_axon_ntff_profile_hook
        mod.get_axon_ntff_profile_hook = get_axon_ntff_profile_hook
        sys.modules["antenv.axon_hooks"] = mod
        antenv.axon_hooks = mod
        mod._hook = _ntff_profile_via_ctypes("/opt/axon/libaxon_pjrt.so")
        return True
    except Exception as e:  # profiling is best-effort
        print(f"NTFF hook install failed: {e}")
        return False


LAST_RESULTS = None


def kernel(cost: np.ndarray, spg: np.ndarray) -> np.ndarray:
    """cost [8,1,48,128,240] f32, spg [8,9,512,960] f32 -> disp1 [8,512,960] f32."""
    global LAST_RESULTS
    cost = np.ascontiguousarray(np.asarray(cost, dtype=np.float32))
    spg = np.ascontiguousarray(np.asarray(spg, dtype=np.float32))
    assert cost.shape == (B, 1, D, H, W) and spg.shape == (B, 9, HF, WF)

    nc = build_program()
    in_maps = [
        {"cost": cost[b, 0], "spg": spg[b]} for b in range(B)
    ]
    trace = bool(int(os.environ.get("KERNEL_TRACE", "0")))
    if trace:
        trace = _install_ntff_hook()
    res = run_bass_kernel_spmd(
        nc, in_maps, core_ids=list(range(N_CORES)), trace=trace
    )
    LAST_RESULTS = res
    out = np.stack([res.results[b]["out"] for b in range(B)], axis=0)
    return out.astype(np.float32, copy=False)


# revision 14
# speedup vs baseline: 1.0691x; 1.0691x over previous
"""Trainium2 Bass kernel for CoExDispProcessor (topk_masking).

Per-sample computation (data-parallel over batch across 8 cores):
  1. top-2 over the D=48 disparity axis of cost [1,48,128,240] -> softmax
     blend of the two indices -> disp4 [128,240]
  2. 3x3 unfold of disp4 (zero pad) -> nearest 4x upsample -> weighted sum
     with softmax over the 9 channels of spg [9,512,960] -> disp1 [512,960]

Kernel layout choices:
  - top-2: cost DMA'd into [128(h), 48(d), wchunk] tiles; vector.max /
    max_index (top-8 values+indices) per w column on strided [128, 48] APs.
  - fine stage: partition = coarse row R (128 of them), free = (dr, w).
    exp on ACT (bf16 out), 9-channel weighted accumulation on DVE with the
    coarse patch values broadcast via stride-0 APs.
"""

import os
import sys
from contextlib import ExitStack

import numpy as np

if "/opt/trn_rl_repo" not in sys.path:
    sys.path.insert(0, "/opt/trn_rl_repo")

import concourse.bass as bass
import concourse.bacc as bacc
import concourse.tile as tile
from concourse import mybir
from concourse.bass_utils import run_bass_kernel_spmd

F32 = mybir.dt.float32
BF16 = mybir.dt.bfloat16
FP16 = mybir.dt.float16
U16 = mybir.dt.uint16
OP = mybir.AluOpType
ACT = mybir.ActivationFunctionType

B, D, H, W = 8, 48, 128, 240
NPIX = H * W  # 30720
HF, WF = 4 * H, 4 * W  # 512, 960
N_CORES = 8

# top-2 phase tiling: ragged chunks of w columns; first three cover the left
# half (128 cols) so the left-half back end can start while the right half
# still streams
COST_CHUNKS = [32, 48, 48, 56, 56]
HALF_W = 128

# fine phase tiling: split the 960 fine columns into chunks
FINE_CHUNK = 480  # fine columns per chunk (= 120 coarse cols)
N_FINE_CHUNKS = WF // FINE_CHUNK  # 4
WC = FINE_CHUNK // 4  # coarse columns per chunk (60)


def _act_reciprocal(nc, out_ap, in_ap):
    eng = nc.scalar
    return eng.add_instruction(
        mybir.InstActivation(
            name=nc.get_next_instruction_name(),
            func=ACT.Reciprocal,
            ins=[
                eng.lower_ap(in_ap),
                mybir.ImmediateValue(dtype=F32, value=0.0),
                mybir.ImmediateValue(dtype=F32, value=1.0),
                mybir.ImmediateValue(dtype=F32, value=0.0),
            ],
            outs=[eng.lower_ap(out_ap)],
        )
    )


def build_kernel(ctx: ExitStack, tc: tile.TileContext, out_d, cost_d, spg_d):
    nc = tc.nc

    cost_hdw = cost_d.transpose([1, 0, 2])  # [128(h), 48(d), 240(w)] view
    spg_v = spg_d.rearrange(
        "c (R dr) (k w) -> c R dr k w", dr=4, k=N_FINE_CHUNKS
    )
    out_v = out_d.rearrange("(R dr) (k w) -> R dr k w", dr=4, k=N_FINE_CHUNKS)

    persist = ctx.enter_context(tc.tile_pool(name="persist", bufs=1))
    cost_pool = ctx.enter_context(tc.tile_pool(name="costc", bufs=3))
    small = ctx.enter_context(tc.tile_pool(name="small", bufs=1))
    raw_pool = ctx.enter_context(tc.tile_pool(name="raw", bufs=3))
    e_pool = ctx.enter_context(tc.tile_pool(name="epool", bufs=2 * 9))
    den_pool = ctx.enter_context(tc.tile_pool(name="den", bufs=2))
    prod_pool = ctx.enter_context(tc.tile_pool(name="prod", bufs=6))
    fin_pool = ctx.enter_context(tc.tile_pool(name="fin", bufs=1))
    out_pool = ctx.enter_context(tc.tile_pool(name="outp", bufs=2))

    # ---- all input DMAs created first: cost chunks, then the spg stream ----
    cost_tiles = []
    w0 = 0
    for nw in COST_CHUNKS:
        ctile = cost_pool.tile([128, D, nw], F32, tag="cost")
        nc.sync.dma_start(ctile[:], cost_hdw[:, :, w0:w0 + nw])
        cost_tiles.append((w0, nw, ctile))
        w0 += nw

    raw_tiles = {}
    for k in range(N_FINE_CHUNKS):
        for c in range(9):
            raw = raw_pool.tile([128, 4, FINE_CHUNK], F32, tag="raw")
            nc.sync.dma_start(raw[:], spg_v[c, :, :, k, :])
            raw_tiles[(k, c)] = raw

    # ---- fine part 1: exp on ACT, denominator accumulation on GpSimd ------
    e_tiles = {}
    den_tiles = []
    for k in range(N_FINE_CHUNKS):
        den = den_pool.tile([128, 4 * FINE_CHUNK], FP16, tag="den")
        den_tiles.append(den)
        for c in range(9):
            e = e_pool.tile([128, 4, FINE_CHUNK], FP16, tag="e")
            e_tiles[(k, c)] = e
            nc.scalar.activation(e[:], raw_tiles[(k, c)][:], ACT.Exp)
            ef = e[:].rearrange("p a b -> p (a b)")
            if c == 0:
                nc.gpsimd.tensor_copy(den[:], ef)
            else:
                nc.gpsimd.tensor_add(den[:], den[:], ef)

    # ---- persistent coarse tiles -----------------------------------------
    v8 = persist.tile([128, W, 8], F32)
    i8 = persist.tile([128, W, 8], U16)
    i1f = small.tile([128, W], F32)
    i2f = small.tile([128, W], F32)
    delta = small.tile([128, W], F32)
    texp = small.tile([128, W], F32)
    numc = small.tile([128, W], F32)
    denc = small.tile([128, W], F32)
    rden = small.tile([128, W], F32)
    disp4 = small.tile([128, W], F32)
    rv = []
    urep = []
    for s in range(3):
        t = small.tile([128, W + 2], F32, tag=f"rv{s}")
        rv.append(t)
        nc.vector.memset(t[:], 0.0)
        u = small.tile([128, 4 * (W + 2)], FP16, tag=f"urep{s}")
        urep.append(u)

    def maxes(ci):
        w0, nw, ctile = cost_tiles[ci]
        for j in range(nw):
            nc.vector.max(out=v8[:, w0 + j], in_=ctile[:, :, j])
        for j in range(nw):
            w = w0 + j
            nc.vector.max_index(i8[:, w], v8[:, w], ctile[:, :, j])

    def disp4_half(h):
        # columns [a, b) of the coarse grid
        a, b = (0, HALF_W) if h == 0 else (HALF_W, W)
        sl = slice(a, b)
        nc.vector.tensor_copy(i1f[:, sl], i8[:, sl, 0])
        nc.vector.tensor_copy(i2f[:, sl], i8[:, sl, 1])
        nc.vector.tensor_sub(delta[:, sl], v8[:, sl, 1], v8[:, sl, 0])
        nc.scalar.activation(texp[:, sl], delta[:, sl], ACT.Exp)
        nc.vector.tensor_mul(numc[:, sl], texp[:, sl], i2f[:, sl])
        nc.vector.tensor_add(numc[:, sl], numc[:, sl], i1f[:, sl])
        nc.vector.tensor_scalar_add(denc[:, sl], texp[:, sl], 1.0)
        nc.vector.reciprocal(rden[:, sl], denc[:, sl])
        nc.vector.tensor_mul(disp4[:, sl], numc[:, sl], rden[:, sl])
        # rv[s][r, 1+w] = disp4[r + s - 1, w] for this half's columns
        nc.vector.tensor_copy(rv[1][:, 1 + a:1 + b], disp4[:, sl])
        nc.sync.dma_start(rv[0][1:128, 1 + a:1 + b], disp4[0:127, sl])
        nc.sync.dma_start(rv[2][0:127, 1 + a:1 + b], disp4[1:128, sl])
        # urep covers rv cols [a .. b+2) for h==1, [0 .. b+1) for h==0
        ua, ub = (0, 4 * (HALF_W + 1)) if h == 0 else (4 * (HALF_W + 1), 4 * (W + 2))
        ra, rb = (0, HALF_W + 1) if h == 0 else (HALF_W + 1, W + 2)
        for s in range(3):
            nc.scalar.copy(
                urep[s][:, ua:ub].rearrange("p (x dw) -> p x dw", dw=4),
                rv[s][:, ra:rb].unsqueeze(2).broadcast_to([128, rb - ra, 4]),
            )

    def fine_part2(k):
        prods = []
        for c in range(9):
            cirow, cj = c // 3, c % 3
            off = 4 * (cj + k * WC)
            u4 = (urep[cirow][:, off:off + FINE_CHUNK]
                  .unsqueeze(1).broadcast_to([128, 4, FINE_CHUNK]))
            ev = e_tiles[(k, c)][:]
            p = prod_pool.tile([128, 4 * FINE_CHUNK], FP16, tag="prod")
            nc.vector.tensor_mul(p[:].rearrange("p (a b) -> p a b", a=4), ev, u4)
            prods.append(p)
        for a, b in ((0, 1), (2, 3), (4, 5), (6, 7)):
            nc.vector.tensor_add(prods[a][:], prods[a][:], prods[b][:])
        for a, b in ((0, 2), (4, 6)):
            nc.vector.tensor_add(prods[a][:], prods[a][:], prods[b][:])
        nc.vector.tensor_add(prods[0][:], prods[0][:], prods[4][:])
        nc.vector.tensor_add(prods[0][:], prods[0][:], prods[8][:])
        num = prods[0]
        den = den_tiles[k]
        r0 = fin_pool.tile([128, 4 * FINE_CHUNK], FP16, tag="r0")
        _act_reciprocal(nc, r0[:], den[:])
        m = fin_pool.tile([128, 4 * FINE_CHUNK], FP16, tag="m")
        nc.vector.tensor_mul(m[:], den[:], r0[:])
        nc.vector.tensor_scalar(m[:], m[:], -1.0, 2.0, op0=OP.mult, op1=OP.add)
        nc.vector.tensor_mul(r0[:], r0[:], m[:])
        outt = out_pool.tile([128, 4, FINE_CHUNK], F32, tag="outt")
        nc.vector.scalar_tensor_tensor(
            outt[:].rearrange("p a b -> p (a b)"), num[:], 4.0, r0[:],
            op0=OP.mult, op1=OP.mult,
        )
        nc.sync.dma_start(out_v[:, :, k, :], outt[:])

    # ---- pipelined schedule: left half then right half -------------------
    maxes(0); maxes(1); maxes(2)          # columns 0..HALF_W
    disp4_half(0)
    maxes(3); maxes(4)                    # columns HALF_W..W
    fine_part2(0)
    disp4_half(1)
    fine_part2(1)


def build_program():
    nc = bacc.Bacc(
        "TRN2",
        target_bir_lowering=False,
        debug=False,
        enable_asserts=False,
        num_devices=N_CORES,
    )
    cost_d = nc.dram_tensor("cost", [D, H, W], F32, kind="ExternalInput").ap()
    spg_d = nc.dram_tensor("spg", [9, HF, WF], F32, kind="ExternalInput").ap()
    out_d = nc.dram_tensor("out", [HF, WF], F32, kind="ExternalOutput").ap()
    with tile.TileContext(nc) as tc:
        with ExitStack() as ctx:
            build_kernel(ctx, tc, out_d, cost_d, spg_d)
    nc.compile()
    return nc


def _install_ntff_hook():
    """Provide antenv.axon_hooks + register the ctypes NTFF profiler.

    The agent image's antenv package lacks axon_hooks, so trn_boot's hook
    registration silently degraded.  Recreate both pieces here.
    """
    import types

    if "antenv.axon_hooks" in sys.modules:
        return True
    try:
        import antenv
        from trn_agent_boot.trn_boot import _ntff_profile_via_ctypes

        mod = types.ModuleType("antenv.axon_hooks")
        mod._hook = None

        def set_axon_ntff_profile_hook(hook):
            mod._hook = hook

        def get_axon_ntff_profile_hook():
            return mod._hook

        mod.set_axon_ntff_profile_hook = set_axon_ntff_profile_hook
        mod.get_axon_ntff_profile_hook = get_axon_ntff_profile_hook
        sys.modules["antenv.axon_hooks"] = mod
        antenv.axon_hooks = mod
        mod._hook = _ntff_profile_via_ctypes("/opt/axon/libaxon_pjrt.so")
        return True
    except Exception as e:  # profiling is best-effort
        print(f"NTFF hook install failed: {e}")
        return False


LAST_RESULTS = None


def kernel(cost: np.ndarray, spg: np.ndarray) -> np.ndarray:
    """cost [8,1,48,128,240] f32, spg [8,9,512,960] f32 -> disp1 [8,512,960] f32."""
    global LAST_RESULTS
    cost = np.ascontiguousarray(np.asarray(cost, dtype=np.float32))
    spg = np.ascontiguousarray(np.asarray(spg, dtype=np.float32))
    assert cost.shape == (B, 1, D, H, W) and spg.shape == (B, 9, HF, WF)

    nc = build_program()
    in_maps = [
        {"cost": cost[b, 0], "spg": spg[b]} for b in range(B)
    ]
    trace = bool(int(os.environ.get("KERNEL_TRACE", "0")))
    if trace:
        trace = _install_ntff_hook()
    res = run_bass_kernel_spmd(
        nc, in_maps, core_ids=list(range(N_CORES)), trace=trace
    )
    LAST_RESULTS = res
    out = np.stack([res.results[b]["out"] for b in range(B)], axis=0)
    return out.astype(np.float32, copy=False)


# revision 15
# speedup vs baseline: 1.1672x; 1.0918x over previous
"""Trainium2 Bass kernel for CoExDispProcessor (topk_masking).

Per-sample computation (data-parallel over batch across 8 cores):
  1. top-2 over the D=48 disparity axis of cost [1,48,128,240] -> softmax
     blend of the two indices -> disp4 [128,240]
  2. 3x3 unfold of disp4 (zero pad) -> nearest 4x upsample -> weighted sum
     with softmax over the 9 channels of spg [9,512,960] -> disp1 [512,960]

Kernel layout choices:
  - top-2: cost DMA'd into [128(h), 48(d), wchunk] tiles; vector.max /
    max_index (top-8 values+indices) per w column on strided [128, 48] APs.
  - fine stage: partition = coarse row R (128 of them), free = (dr, w).
    exp on ACT (bf16 out), 9-channel weighted accumulation on DVE with the
    coarse patch values broadcast via stride-0 APs.
"""

import os
import sys
from contextlib import ExitStack

import numpy as np

if "/opt/trn_rl_repo" not in sys.path:
    sys.path.insert(0, "/opt/trn_rl_repo")

import concourse.bass as bass
import concourse.bacc as bacc
import concourse.tile as tile
from concourse import mybir
from concourse.bass_utils import run_bass_kernel_spmd

F32 = mybir.dt.float32
BF16 = mybir.dt.bfloat16
FP16 = mybir.dt.float16
U16 = mybir.dt.uint16
OP = mybir.AluOpType
ACT = mybir.ActivationFunctionType

B, D, H, W = 8, 48, 128, 240
NPIX = H * W  # 30720
HF, WF = 4 * H, 4 * W  # 512, 960
N_CORES = 8

# top-2 phase tiling: ragged chunks of w columns; first three cover the left
# half (128 cols) so the left-half back end can start while the right half
# still streams
COST_CHUNKS = [32, 48, 48, 56, 56]
HALF_W = 128

# fine phase tiling: split the 960 fine columns into chunks
FINE_CHUNK = 480  # fine columns per chunk (= 120 coarse cols)
N_FINE_CHUNKS = WF // FINE_CHUNK  # 4
WC = FINE_CHUNK // 4  # coarse columns per chunk (60)


def _act_reciprocal(nc, out_ap, in_ap):
    eng = nc.scalar
    return eng.add_instruction(
        mybir.InstActivation(
            name=nc.get_next_instruction_name(),
            func=ACT.Reciprocal,
            ins=[
                eng.lower_ap(in_ap),
                mybir.ImmediateValue(dtype=F32, value=0.0),
                mybir.ImmediateValue(dtype=F32, value=1.0),
                mybir.ImmediateValue(dtype=F32, value=0.0),
            ],
            outs=[eng.lower_ap(out_ap)],
        )
    )


def build_kernel(ctx: ExitStack, tc: tile.TileContext, out_d, cost_d, spg_d):
    nc = tc.nc

    cost_hdw = cost_d.transpose([1, 0, 2])  # [128(h), 48(d), 240(w)] view
    spg_v = spg_d.rearrange(
        "c (R dr) (k w) -> c R dr k w", dr=4, k=N_FINE_CHUNKS
    )
    out_v = out_d.rearrange("(R dr) (k w) -> R dr k w", dr=4, k=N_FINE_CHUNKS)

    persist = ctx.enter_context(tc.tile_pool(name="persist", bufs=1))
    cost_pool = ctx.enter_context(tc.tile_pool(name="costc", bufs=3))
    small = ctx.enter_context(tc.tile_pool(name="small", bufs=1))
    raw_pool = ctx.enter_context(tc.tile_pool(name="raw", bufs=3))
    e_pool = ctx.enter_context(tc.tile_pool(name="epool", bufs=2 * 9))
    den_pool = ctx.enter_context(tc.tile_pool(name="den", bufs=2))
    prod_pool = ctx.enter_context(tc.tile_pool(name="prod", bufs=6))
    fin_pool = ctx.enter_context(tc.tile_pool(name="fin", bufs=1))
    out_pool = ctx.enter_context(tc.tile_pool(name="outp", bufs=2))

    # ---- all input DMAs created first: cost chunks, then the spg stream ----
    cost_tiles = []
    w0 = 0
    for nw in COST_CHUNKS:
        ctile = cost_pool.tile([128, D, nw], F32, tag="cost")
        nc.sync.dma_start(ctile[:], cost_hdw[:, :, w0:w0 + nw])
        cost_tiles.append((w0, nw, ctile))
        w0 += nw

    raw_tiles = {}
    for k in range(N_FINE_CHUNKS):
        for c in range(9):
            raw = raw_pool.tile([128, 4, FINE_CHUNK], F32, tag="raw")
            nc.scalar.dma_start(raw[:], spg_v[c, :, :, k, :])
            raw_tiles[(k, c)] = raw

    # ---- fine part 1: exp on ACT, denominator accumulation on GpSimd ------
    e_tiles = {}
    den_tiles = []
    for k in range(N_FINE_CHUNKS):
        den = den_pool.tile([128, 4 * FINE_CHUNK], FP16, tag="den")
        den_tiles.append(den)
        for c in range(9):
            e = e_pool.tile([128, 4, FINE_CHUNK], FP16, tag="e")
            e_tiles[(k, c)] = e
            nc.scalar.activation(e[:], raw_tiles[(k, c)][:], ACT.Exp)
            ef = e[:].rearrange("p a b -> p (a b)")
            if c == 0:
                nc.gpsimd.tensor_copy(den[:], ef)
            else:
                nc.gpsimd.tensor_add(den[:], den[:], ef)

    # ---- persistent coarse tiles -----------------------------------------
    v8 = persist.tile([128, W, 8], F32)
    i8 = persist.tile([128, W, 8], U16)
    i1f = small.tile([128, W], F32)
    i2f = small.tile([128, W], F32)
    delta = small.tile([128, W], F32)
    texp = small.tile([128, W], F32)
    numc = small.tile([128, W], F32)
    denc = small.tile([128, W], F32)
    rden = small.tile([128, W], F32)
    disp4 = small.tile([128, W], F32)
    rv = []
    urep = []
    for s in range(3):
        t = small.tile([128, W + 2], F32, tag=f"rv{s}")
        rv.append(t)
        nc.vector.memset(t[:], 0.0)
        u = small.tile([128, 4 * (W + 2)], FP16, tag=f"urep{s}")
        urep.append(u)

    def maxes(ci):
        w0, nw, ctile = cost_tiles[ci]
        for j in range(nw):
            nc.vector.max(out=v8[:, w0 + j], in_=ctile[:, :, j])
        for j in range(nw):
            w = w0 + j
            nc.vector.max_index(i8[:, w], v8[:, w], ctile[:, :, j])

    def disp4_half(h):
        # columns [a, b) of the coarse grid
        a, b = (0, HALF_W) if h == 0 else (HALF_W, W)
        sl = slice(a, b)
        nc.vector.tensor_copy(i1f[:, sl], i8[:, sl, 0])
        nc.vector.tensor_copy(i2f[:, sl], i8[:, sl, 1])
        nc.vector.tensor_sub(delta[:, sl], v8[:, sl, 1], v8[:, sl, 0])
        nc.scalar.activation(texp[:, sl], delta[:, sl], ACT.Exp)
        nc.vector.tensor_mul(numc[:, sl], texp[:, sl], i2f[:, sl])
        nc.vector.tensor_add(numc[:, sl], numc[:, sl], i1f[:, sl])
        nc.vector.tensor_scalar_add(denc[:, sl], texp[:, sl], 1.0)
        nc.vector.reciprocal(rden[:, sl], denc[:, sl])
        nc.vector.tensor_mul(disp4[:, sl], numc[:, sl], rden[:, sl])
        # rv[s][r, 1+w] = disp4[r + s - 1, w] for this half's columns
        nc.vector.tensor_copy(rv[1][:, 1 + a:1 + b], disp4[:, sl])
        nc.sync.dma_start(rv[0][1:128, 1 + a:1 + b], disp4[0:127, sl])
        nc.sync.dma_start(rv[2][0:127, 1 + a:1 + b], disp4[1:128, sl])
        # urep covers rv cols [a .. b+2) for h==1, [0 .. b+1) for h==0
        ua, ub = (0, 4 * (HALF_W + 1)) if h == 0 else (4 * (HALF_W + 1), 4 * (W + 2))
        ra, rb = (0, HALF_W + 1) if h == 0 else (HALF_W + 1, W + 2)
        for s in range(3):
            nc.scalar.copy(
                urep[s][:, ua:ub].rearrange("p (x dw) -> p x dw", dw=4),
                rv[s][:, ra:rb].unsqueeze(2).broadcast_to([128, rb - ra, 4]),
            )

    def fine_part2(k):
        prods = []
        for c in range(9):
            cirow, cj = c // 3, c % 3
            off = 4 * (cj + k * WC)
            u4 = (urep[cirow][:, off:off + FINE_CHUNK]
                  .unsqueeze(1).broadcast_to([128, 4, FINE_CHUNK]))
            ev = e_tiles[(k, c)][:]
            p = prod_pool.tile([128, 4 * FINE_CHUNK], FP16, tag="prod")
            nc.vector.tensor_mul(p[:].rearrange("p (a b) -> p a b", a=4), ev, u4)
            prods.append(p)
        for a, b in ((0, 1), (2, 3), (4, 5), (6, 7)):
            nc.vector.tensor_add(prods[a][:], prods[a][:], prods[b][:])
        for a, b in ((0, 2), (4, 6)):
            nc.vector.tensor_add(prods[a][:], prods[a][:], prods[b][:])
        nc.vector.tensor_add(prods[0][:], prods[0][:], prods[4][:])
        nc.vector.tensor_add(prods[0][:], prods[0][:], prods[8][:])
        num = prods[0]
        den = den_tiles[k]
        r0 = fin_pool.tile([128, 4 * FINE_CHUNK], FP16, tag="r0")
        _act_reciprocal(nc, r0[:], den[:])
        m = fin_pool.tile([128, 4 * FINE_CHUNK], FP16, tag="m")
        nc.vector.tensor_mul(m[:], den[:], r0[:])
        nc.vector.tensor_scalar(m[:], m[:], -1.0, 2.0, op0=OP.mult, op1=OP.add)
        nc.vector.tensor_mul(r0[:], r0[:], m[:])
        outt = out_pool.tile([128, 4, FINE_CHUNK], F32, tag="outt")
        nc.vector.scalar_tensor_tensor(
            outt[:].rearrange("p a b -> p (a b)"), num[:], 4.0, r0[:],
            op0=OP.mult, op1=OP.mult,
        )
        nc.sync.dma_start(out_v[:, :, k, :], outt[:])

    # ---- pipelined schedule: left half then right half -------------------
    maxes(0); maxes(1); maxes(2)          # columns 0..HALF_W
    disp4_half(0)
    maxes(3); maxes(4)                    # columns HALF_W..W
    fine_part2(0)
    disp4_half(1)
    fine_part2(1)


def build_program():
    nc = bacc.Bacc(
        "TRN2",
        target_bir_lowering=False,
        debug=False,
        enable_asserts=False,
        num_devices=N_CORES,
    )
    cost_d = nc.dram_tensor("cost", [D, H, W], F32, kind="ExternalInput").ap()
    spg_d = nc.dram_tensor("spg", [9, HF, WF], F32, kind="ExternalInput").ap()
    out_d = nc.dram_tensor("out", [HF, WF], F32, kind="ExternalOutput").ap()
    with tile.TileContext(nc) as tc:
        with ExitStack() as ctx:
            build_kernel(ctx, tc, out_d, cost_d, spg_d)
    nc.compile()
    return nc


def _install_ntff_hook():
    """Provide antenv.axon_hooks + register the ctypes NTFF profiler.

    The agent image's antenv package lacks axon_hooks, so trn_boot's hook
    registration silently degraded.  Recreate both pieces here.
    """
    import types

    if "antenv.axon_hooks" in sys.modules:
        return True
    try:
        import antenv
        from trn_agent_boot.trn_boot import _ntff_profile_via_ctypes

        mod = types.ModuleType("antenv.axon_hooks")
        mod._hook = None

        def set_axon_ntff_profile_hook(hook):
            mod._hook = hook

        def get_axon_ntff_profile_hook():
            return mod._hook

        mod.set_axon_ntff_profile_hook = set_axon_ntff_profile_hook
        mod.get_axon_ntff_profile_hook = get_axon_ntff_profile_hook
        sys.modules["antenv.axon_hooks"] = mod
        antenv.axon_hooks = mod
        mod._hook = _ntff_profile_via_ctypes("/opt/axon/libaxon_pjrt.so")
        return True
    except Exception as e:  # profiling is best-effort
        print(f"NTFF hook install failed: {e}")
        return False


LAST_RESULTS = None


def kernel(cost: np.ndarray, spg: np.ndarray) -> np.ndarray:
    """cost [8,1,48,128,240] f32, spg [8,9,512,960] f32 -> disp1 [8,512,960] f32."""
    global LAST_RESULTS
    cost = np.ascontiguousarray(np.asarray(cost, dtype=np.float32))
    spg = np.ascontiguousarray(np.asarray(spg, dtype=np.float32))
    assert cost.shape == (B, 1, D, H, W) and spg.shape == (B, 9, HF, WF)

    nc = build_program()
    in_maps = [
        {"cost": cost[b, 0], "spg": spg[b]} for b in range(B)
    ]
    trace = bool(int(os.environ.get("KERNEL_TRACE", "0")))
    if trace:
        trace = _install_ntff_hook()
    res = run_bass_kernel_spmd(
        nc, in_maps, core_ids=list(range(N_CORES)), trace=trace
    )
    LAST_RESULTS = res
    out = np.stack([res.results[b]["out"] for b in range(B)], axis=0)
    return out.astype(np.float32, copy=False)
